# revision 8
# baseline (speedup 1.0000x reference)
"""LIF (leaky integrate-and-fire) forward pass on 8 Trainium2 NeuronCores.

Problem: input_current [T=64, B=32, N=16384] f32.
    v <- v*beta + I_t ; spike = (v > v_th) ; v <- v_reset where spiked.
Returns (spikes [T,B,N] f32, v_final [B,N] f32).

Sharding: embarrassingly parallel over batch — B=32 split as 4 batches per
core across 8 cores. Each core's slice input_current[:, c*4:(c+1)*4, :] is
contiguous in DRAM per timestep (256 KiB slabs).

Per-core layout: state v is [128 partitions, 512 free] f32 (4*16384 = 65536
elements). Per timestep, 3 DVE ops carry the recurrence:
    1. scalar_tensor_tensor: v = (v * beta) + I_t
    2. tensor_scalar is_gt:  s = (v > v_th)  -> 1.0/0.0
    3. copy_predicated:      v = v_reset where s
DMAs are grouped G timesteps per transfer (>= 1 MiB) for bandwidth.
"""

import numpy as np

T, B, N = 64, 32, 16384
N_CORES = 8
B_PER = B // N_CORES  # 4 batches per core
P = 128
F = (B_PER * N) // P  # 512 free elements per partition
G = 4  # timesteps per DMA group (4 * 256 KiB = 1 MiB per transfer)


def build_program(beta: float, v_th: float, v_reset: float):
    """Build the per-core Bass program (identical across cores, SPMD)."""
    import concourse.tile as tile
    from concourse import bacc, mybir

    nc = bacc.Bacc(None)
    f32 = mybir.dt.float32

    x = nc.declare_dram_parameter("x", [T, B_PER, N], f32, isOutput=False)
    spikes = nc.declare_dram_parameter("spikes", [T, B_PER, N], f32, isOutput=True)
    v_final = nc.declare_dram_parameter("v_final", [B_PER, N], f32, isOutput=True)

    # [T, B_PER, N] -> [128, T, 512]; flat (b,n) index e maps to
    # partition e//512, free e%512. Partition-first so the DMA APs
    # enumerate (p, t, f) on both the DRAM and SBUF sides.
    x_r = x[:].rearrange("t b (q f) -> (b q) t f", f=F)
    s_r = spikes[:].rearrange("t b (q f) -> (b q) t f", f=F)
    vf_r = v_final[:].rearrange("b (q f) -> (b q) f", f=F)

    mult = mybir.AluOpType.mult
    add = mybir.AluOpType.add
    is_gt = mybir.AluOpType.is_gt

    u8 = mybir.dt.uint8
    is_le = mybir.AluOpType.is_le
    fast_reset = (v_reset == 0.0)

    with tile.TileContext(nc) as tc:
        with (
            tc.tile_pool(name="state", bufs=1) as state_pool,
            tc.tile_pool(name="vtmp", bufs=3) as tmp_pool,
            tc.tile_pool(name="xin", bufs=4) as in_pool,
            tc.tile_pool(name="sout", bufs=4) as out_pool,
            tc.tile_pool(name="mask", bufs=4) as mask_pool,
        ):
            v = state_pool.tile([P, F], f32)
            nc.vector.memset(v[:], 0.0)
            if not fast_reset:
                rst = state_pool.tile([P, F], f32)
                nc.vector.memset(rst[:], v_reset)

            for t0 in range(0, T, G):
                xg = in_pool.tile([P, G * F], f32)
                nc.sync.dma_start(
                    out=xg[:].rearrange("p (t f) -> p t f", t=G),
                    in_=x_r[:, t0 : t0 + G],
                )
                sg = out_pool.tile([P, G * F], f32)
                for k in range(G):
                    xt = xg[:, k * F : (k + 1) * F]
                    st = sg[:, k * F : (k + 1) * F]
                    if fast_reset:
                        # Critical chain is 2 DVE ops; spike output is
                        # computed off-chain on the idle GpSimd engine.
                        vt = tmp_pool.tile([P, F], f32)
                        # v' = (v * beta) + I_t
                        nc.vector.scalar_tensor_tensor(
                            out=vt[:], in0=v[:], scalar=beta, in1=xt,
                            op0=mult, op1=add,
                        )
                        # spikes = (v' > v_th) -> 1.0/0.0, straight to f32
                        nc.gpsimd.tensor_scalar(st, vt[:], v_th, None, is_gt)
                        # v = (v' <= v_th) * v'  == hard reset to 0
                        nc.vector.scalar_tensor_tensor(
                            out=v[:], in0=vt[:], scalar=v_th, in1=vt[:],
                            op0=is_le, op1=mult,
                        )
                    else:
                        # general v_reset path
                        nc.vector.scalar_tensor_tensor(
                            out=v[:], in0=v[:], scalar=beta, in1=xt,
                            op0=mult, op1=add,
                        )
                        m = mask_pool.tile([P, F], u8)
                        nc.vector.tensor_scalar(m[:], v[:], v_th, None, is_gt)
                        nc.vector.copy_predicated(v[:], m[:], rst[:])
                        nc.scalar.copy(st, m[:])
                nc.sync.dma_start(
                    out=s_r[:, t0 : t0 + G],
                    in_=sg[:].rearrange("p (t f) -> p t f", t=G),
                )
            nc.sync.dma_start(out=vf_r, in_=v[:])
    nc.finalize()
    return nc


def shard_inputs(input_current: np.ndarray) -> list[dict]:
    input_current = np.ascontiguousarray(input_current, dtype=np.float32)
    return [
        {"x": input_current[:, c * B_PER : (c + 1) * B_PER, :]}
        for c in range(N_CORES)
    ]


def gather_outputs(results: list[dict]):
    spikes = np.concatenate([r["spikes"] for r in results], axis=1)
    v_final = np.concatenate([r["v_final"] for r in results], axis=0)
    return spikes, v_final


def kernel(input_current, beta, v_th, v_reset, k_superspike=None):
    from concourse.bass_utils import run_bass_kernel_spmd

    nc = build_program(float(beta), float(v_th), float(v_reset))
    in_maps = shard_inputs(np.asarray(input_current))
    res = run_bass_kernel_spmd(nc, in_maps, list(range(N_CORES)))
    return gather_outputs(res.results)


# revision 11
# speedup vs baseline: 4.1938x; 4.1938x over previous
"""LIF (leaky integrate-and-fire) forward pass on 8 Trainium2 NeuronCores.

Problem: input_current [T=64, B=32, N=16384] f32.
    v <- v*beta + I_t ; spike = (v > v_th) ; v <- v_reset where spiked.
Returns (spikes [T,B,N] f32, v_final [B,N] f32).

Sharding: embarrassingly parallel over batch — B=32 split as 4 batches per
core across 8 cores. Each core's slice input_current[:, c*4:(c+1)*4, :] is
contiguous in DRAM per timestep (256 KiB slabs).

Per-core layout: state v is [128 partitions, 512 free] f32 (4*16384 = 65536
elements). Per timestep, 3 DVE ops carry the recurrence:
    1. scalar_tensor_tensor: v = (v * beta) + I_t
    2. tensor_scalar is_gt:  s = (v > v_th)  -> 1.0/0.0
    3. copy_predicated:      v = v_reset where s
DMAs are grouped G timesteps per transfer (>= 1 MiB) for bandwidth.
"""

import numpy as np

T, B, N = 64, 32, 16384
N_CORES = 8
B_PER = B // N_CORES  # 4 batches per core
P = 128
F = (B_PER * N) // P  # 512 free elements per partition
G = 4  # timesteps per DMA group (4 * 256 KiB = 1 MiB per transfer)


def build_program(beta: float, v_th: float, v_reset: float):
    """Build the per-core Bass program (identical across cores, SPMD)."""
    import concourse.tile as tile
    from concourse import bacc, mybir

    nc = bacc.Bacc(None)
    f32 = mybir.dt.float32

    x = nc.declare_dram_parameter("x", [T, B_PER, N], f32, isOutput=False)
    spikes = nc.declare_dram_parameter("spikes", [T, B_PER, N], f32, isOutput=True)
    v_final = nc.declare_dram_parameter("v_final", [B_PER, N], f32, isOutput=True)

    # [T, B_PER, N] -> [128, T, 512]; flat (b,n) index e maps to
    # partition e//512, free e%512. Partition-first so the DMA APs
    # enumerate (p, t, f) on both the DRAM and SBUF sides.
    x_r = x[:].rearrange("t b (q f) -> (b q) t f", f=F)
    s_r = spikes[:].rearrange("t b (q f) -> (b q) t f", f=F)
    vf_r = v_final[:].rearrange("b (q f) -> (b q) f", f=F)

    mult = mybir.AluOpType.mult
    add = mybir.AluOpType.add
    is_gt = mybir.AluOpType.is_gt

    u8 = mybir.dt.uint8
    is_le = mybir.AluOpType.is_le
    fast_reset = (v_reset == 0.0)

    with tile.TileContext(nc) as tc:
        with (
            tc.tile_pool(name="state", bufs=1) as state_pool,
            tc.tile_pool(name="vtmp", bufs=3) as tmp_pool,
            tc.tile_pool(name="xin", bufs=4) as in_pool,
            tc.tile_pool(name="sout", bufs=4) as out_pool,
            tc.tile_pool(name="mask", bufs=4) as mask_pool,
        ):
            v = state_pool.tile([P, F], f32)
            nc.vector.memset(v[:], 0.0)
            if fast_reset:
                nthr = state_pool.tile([P, 1], f32)
                nc.vector.memset(nthr[:], -v_th)
            else:
                rst = state_pool.tile([P, F], f32)
                nc.vector.memset(rst[:], v_reset)

            for t0 in range(0, T, G):
                xg = in_pool.tile([P, G * F], f32)
                nc.sync.dma_start(
                    out=xg[:].rearrange("p (t f) -> p t f", t=G),
                    in_=x_r[:, t0 : t0 + G],
                )
                sg = out_pool.tile([P, G * F], f32)
                for k in range(G):
                    xt = xg[:, k * F : (k + 1) * F]
                    st = sg[:, k * F : (k + 1) * F]
                    if fast_reset:
                        # Critical chain is 2 DVE ops; spike output is
                        # computed off-chain on the Scalar (ACT) engine:
                        # spikes = Relu(Sign(v' - v_th)) -> exact 1.0/0.0
                        # (sign(0)=0, matching the strict > of the reference).
                        vt = tmp_pool.tile([P, F], f32)
                        # v' = (v * beta) + I_t
                        nc.vector.scalar_tensor_tensor(
                            out=vt[:], in0=v[:], scalar=beta, in1=xt,
                            op0=mult, op1=add,
                        )
                        sgn = mask_pool.tile([P, F], f32)
                        nc.scalar.activation(
                            sgn[:], vt[:], mybir.ActivationFunctionType.Sign,
                            bias=nthr[:],
                        )
                        nc.scalar.activation(
                            st, sgn[:], mybir.ActivationFunctionType.Relu,
                        )
                        # v = (v' <= v_th) * v'  == hard reset to 0
                        nc.vector.scalar_tensor_tensor(
                            out=v[:], in0=vt[:], scalar=v_th, in1=vt[:],
                            op0=is_le, op1=mult,
                        )
                    else:
                        # general v_reset path
                        nc.vector.scalar_tensor_tensor(
                            out=v[:], in0=v[:], scalar=beta, in1=xt,
                            op0=mult, op1=add,
                        )
                        m = mask_pool.tile([P, F], u8)
                        nc.vector.tensor_scalar(m[:], v[:], v_th, None, is_gt)
                        nc.vector.copy_predicated(v[:], m[:], rst[:])
                        nc.scalar.copy(st, m[:])
                nc.sync.dma_start(
                    out=s_r[:, t0 : t0 + G],
                    in_=sg[:].rearrange("p (t f) -> p t f", t=G),
                )
            nc.sync.dma_start(out=vf_r, in_=v[:])
    nc.finalize()
    return nc


def shard_inputs(input_current: np.ndarray) -> list[dict]:
    input_current = np.ascontiguousarray(input_current, dtype=np.float32)
    return [
        {"x": input_current[:, c * B_PER : (c + 1) * B_PER, :]}
        for c in range(N_CORES)
    ]


def gather_outputs(results: list[dict]):
    spikes = np.concatenate([r["spikes"] for r in results], axis=1)
    v_final = np.concatenate([r["v_final"] for r in results], axis=0)
    return spikes, v_final


def kernel(input_current, beta, v_th, v_reset, k_superspike=None):
    from concourse.bass_utils import run_bass_kernel_spmd

    nc = build_program(float(beta), float(v_th), float(v_reset))
    in_maps = shard_inputs(np.asarray(input_current))
    res = run_bass_kernel_spmd(nc, in_maps, list(range(N_CORES)))
    return gather_outputs(res.results)


# revision 12
# speedup vs baseline: 4.2103x; 1.0039x over previous
"""LIF (leaky integrate-and-fire) forward pass on 8 Trainium2 NeuronCores.

Problem: input_current [T=64, B=32, N=16384] f32.
    v <- v*beta + I_t ; spike = (v > v_th) ; v <- v_reset where spiked.
Returns (spikes [T,B,N] f32, v_final [B,N] f32).

Sharding: embarrassingly parallel over batch — B=32 split as 4 batches per
core across 8 cores. Each core's slice input_current[:, c*4:(c+1)*4, :] is
contiguous in DRAM per timestep (256 KiB slabs).

Per-core layout: state v is [128 partitions, 512 free] f32 (4*16384 = 65536
elements). Per timestep, 3 DVE ops carry the recurrence:
    1. scalar_tensor_tensor: v = (v * beta) + I_t
    2. tensor_scalar is_gt:  s = (v > v_th)  -> 1.0/0.0
    3. copy_predicated:      v = v_reset where s
DMAs are grouped G timesteps per transfer (>= 1 MiB) for bandwidth.
"""

import numpy as np

T, B, N = 64, 32, 16384
N_CORES = 8
B_PER = B // N_CORES  # 4 batches per core
P = 128
F = (B_PER * N) // P  # 512 free elements per partition
G = 4  # timesteps per DMA group (4 * 256 KiB = 1 MiB per transfer)


def build_program(beta: float, v_th: float, v_reset: float):
    """Build the per-core Bass program (identical across cores, SPMD)."""
    import concourse.tile as tile
    from concourse import bacc, mybir

    nc = bacc.Bacc(None)
    f32 = mybir.dt.float32

    x = nc.declare_dram_parameter("x", [T, B_PER, N], f32, isOutput=False)
    spikes = nc.declare_dram_parameter("spikes", [T, B_PER, N], f32, isOutput=True)
    v_final = nc.declare_dram_parameter("v_final", [B_PER, N], f32, isOutput=True)

    # [T, B_PER, N] -> [128, T, 512]; flat (b,n) index e maps to
    # partition e//512, free e%512. Partition-first so the DMA APs
    # enumerate (p, t, f) on both the DRAM and SBUF sides.
    x_r = x[:].rearrange("t b (q f) -> (b q) t f", f=F)
    s_r = spikes[:].rearrange("t b (q f) -> (b q) t f", f=F)
    vf_r = v_final[:].rearrange("b (q f) -> (b q) f", f=F)

    mult = mybir.AluOpType.mult
    add = mybir.AluOpType.add
    is_gt = mybir.AluOpType.is_gt

    u8 = mybir.dt.uint8
    is_le = mybir.AluOpType.is_le
    fast_reset = (v_reset == 0.0)

    with tile.TileContext(nc) as tc:
        with (
            tc.tile_pool(name="state", bufs=1) as state_pool,
            tc.tile_pool(name="vtmp", bufs=6) as tmp_pool,
            tc.tile_pool(name="xin", bufs=6) as in_pool,
            tc.tile_pool(name="sout", bufs=6) as out_pool,
            tc.tile_pool(name="mask", bufs=8) as mask_pool,
        ):
            v = state_pool.tile([P, F], f32)
            nc.vector.memset(v[:], 0.0)
            if fast_reset:
                nthr = state_pool.tile([P, 1], f32)
                nc.vector.memset(nthr[:], -v_th)
            else:
                rst = state_pool.tile([P, F], f32)
                nc.vector.memset(rst[:], v_reset)

            for t0 in range(0, T, G):
                xg = in_pool.tile([P, G * F], f32)
                nc.sync.dma_start(
                    out=xg[:].rearrange("p (t f) -> p t f", t=G),
                    in_=x_r[:, t0 : t0 + G],
                )
                sg = out_pool.tile([P, G * F], f32)
                for k in range(G):
                    xt = xg[:, k * F : (k + 1) * F]
                    st = sg[:, k * F : (k + 1) * F]
                    if fast_reset:
                        # Critical chain is 2 DVE ops; spike output is
                        # computed off-chain on the Scalar (ACT) engine:
                        # spikes = Relu(Sign(v' - v_th)) -> exact 1.0/0.0
                        # (sign(0)=0, matching the strict > of the reference).
                        vt = tmp_pool.tile([P, F], f32)
                        # v' = (v * beta) + I_t
                        nc.vector.scalar_tensor_tensor(
                            out=vt[:], in0=v[:], scalar=beta, in1=xt,
                            op0=mult, op1=add,
                        )
                        sgn = mask_pool.tile([P, F], f32)
                        nc.scalar.activation(
                            sgn[:], vt[:], mybir.ActivationFunctionType.Sign,
                            bias=nthr[:],
                        )
                        nc.scalar.activation(
                            st, sgn[:], mybir.ActivationFunctionType.Relu,
                        )
                        # v = (v' <= v_th) * v'  == hard reset to 0
                        nc.vector.scalar_tensor_tensor(
                            out=v[:], in0=vt[:], scalar=v_th, in1=vt[:],
                            op0=is_le, op1=mult,
                        )
                    else:
                        # general v_reset path
                        nc.vector.scalar_tensor_tensor(
                            out=v[:], in0=v[:], scalar=beta, in1=xt,
                            op0=mult, op1=add,
                        )
                        m = mask_pool.tile([P, F], u8)
                        nc.vector.tensor_scalar(m[:], v[:], v_th, None, is_gt)
                        nc.vector.copy_predicated(v[:], m[:], rst[:])
                        nc.scalar.copy(st, m[:])
                nc.sync.dma_start(
                    out=s_r[:, t0 : t0 + G],
                    in_=sg[:].rearrange("p (t f) -> p t f", t=G),
                )
            nc.sync.dma_start(out=vf_r, in_=v[:])
    nc.finalize()
    return nc


def shard_inputs(input_current: np.ndarray) -> list[dict]:
    input_current = np.ascontiguousarray(input_current, dtype=np.float32)
    return [
        {"x": input_current[:, c * B_PER : (c + 1) * B_PER, :]}
        for c in range(N_CORES)
    ]


def gather_outputs(results: list[dict]):
    spikes = np.concatenate([r["spikes"] for r in results], axis=1)
    v_final = np.concatenate([r["v_final"] for r in results], axis=0)
    return spikes, v_final


def kernel(input_current, beta, v_th, v_reset, k_superspike=None):
    from concourse.bass_utils import run_bass_kernel_spmd

    nc = build_program(float(beta), float(v_th), float(v_reset))
    in_maps = shard_inputs(np.asarray(input_current))
    res = run_bass_kernel_spmd(nc, in_maps, list(range(N_CORES)))
    return gather_outputs(res.results)
